# revision 41
# baseline (speedup 1.0000x reference)
"""MoE routing kernel (top-2 of 32 experts, dense-mix form) for 8 TRN2 cores.

Math identity used: out = sum_e mix_w[:, e] * (x @ W_e) + mix_b @ expert_biases,
where mix_w / mix_b are the dense top-2 softmax mixture coefficients from the
two routers. Experts are sharded 4-per-core; each core computes a partial sum
(its 4 experts plus its 4 experts' bias rows) and the host adds the 8 partials.

The workload is HBM-bound (~9 MB of weight/x traffic per core per iteration;
all 8 cores together sit at the device HBM roofline), so the kernel is built
around one uninterrupted full-rate weight stream:
- Weights are staged host-side in a half-split layout [EPC, 2, 128, KT, HD]
  so every weight DMA has 8 KB-contiguous partition lines; HWDGE descriptor
  generation stays ~0.6 us per 1 MB chunk and never throttles the stream.
- The SP HWDGE ring carries ONLY the input stream (x + weights); outputs and
  small inputs ride the ACT ring, so consecutive reps chain with no gap.
- Router logits are computed exactly from bf16 pieces (x = xtb + xrb residual
  decomposition against rw = rwb + rwr): logits = xtb@[rwb|rwr] + xrb@rwb,
  correct to ~1e-5 (top-2 margins are ~1e-3), saving the f32 x copy.
- The mix-coefficient matmuls (transpose + per-core selection + bias term)
  are emitted at high priority so the in-order PE runs them as soon as the
  softmax lands instead of behind weight-gated expert matmul blocks.
- The last expert's second half streams in shrinking chunks so the final
  matmul + mix-accumulate + output DMA tail after the last byte is minimal.
"""

import sys

if "/opt/trn_rl_repo" not in sys.path:
    sys.path.insert(0, "/opt/trn_rl_repo")

from contextlib import ExitStack

import ml_dtypes
import numpy as np

import concourse.bacc as bacc
import concourse.tile as tile
from concourse import mybir
from concourse.bass_utils import run_bass_kernel_spmd
from concourse.masks import make_identity

B = 128        # batch
D = 1024       # in = out features
E = 32         # experts
NCORES = 8
EPC = E // NCORES   # experts per core
KT = D // 128       # k-tiles of 128 along contraction dim
HD = 512            # psum-bank-sized output chunk

F32 = mybir.dt.float32
BF16 = mybir.dt.bfloat16
ALU = mybir.AluOpType
ACTF = mybir.ActivationFunctionType


def _ctile(pool, name, shape, dtype):
    # unique tag => dedicated slot, never rotated/reused
    return pool.tile(shape, dtype, name=name, tag=name)


def build_program(reps=1):
    # no partition_id: per-core behavior lives entirely in the input data
    # (sel matrix / weight shards), and dropping the tensor removes the
    # per-engine TENSOR_LOADs from the NEFF preamble
    nc = bacc.Bacc("TRN2", enable_partition_id=False)

    # x^T bf16 and its bf16 residual (x - xtb), for exact-enough router logits
    xtb_d = nc.dram_tensor("xtb", [128, KT, B], BF16, kind="ExternalInput")
    xrb_d = nc.dram_tensor("xrb", [128, KT, B], BF16, kind="ExternalInput")
    # router pair: cols 0:64 = bf16(rw), cols 64:128 = bf16(rw - bf16(rw))
    # for rw = [router_w | bias_router_w]
    rwp_d = nc.dram_tensor("rwp", [128, KT, 4 * E], BF16, kind="ExternalInput")
    # weights half-split: whs[e, h, p, k, :] = W_e[k*128+p, h*HD:(h+1)*HD]
    whs_d = nc.dram_tensor("whs", [EPC, 2, 128, KT, HD], BF16, kind="ExternalInput")
    bscl_d = nc.dram_tensor("bscl", [EPC, D], BF16, kind="ExternalInput")
    sel_d = nc.dram_tensor("sel", [2 * E, 2 * EPC], F32, kind="ExternalInput")
    out_d = nc.dram_tensor("out", [B, D], BF16, kind="ExternalOutput")

    with ExitStack() as ctx:
        tc = ctx.enter_context(tile.TileContext(nc))
        const = ctx.enter_context(tc.tile_pool(name="const", bufs=1))
        xin = ctx.enter_context(tc.tile_pool(name="xin", bufs=2))
        # 1.5 reps of weight buffering: rep n+1's weight DMAs must never
        # WAR-wait on rep n's expert matmuls, or the wire stalls whenever
        # the PE lags (and the resulting PE idle re-throttles the HAM)
        wpool = ctx.enter_context(tc.tile_pool(name="wts", bufs=3 * EPC))
        ps_small = ctx.enter_context(tc.tile_pool(name="ps", bufs=1, space="PSUM"))
        ps_e = ctx.enter_context(tc.tile_pool(name="pe", bufs=7, space="PSUM"))

        ident = _ctile(const, "ident", [128, 128], F32)
        make_identity(nc, ident[:])

        def make_head():
            # tiles for a rep's stream head: x pieces + both expert 0 halves
            xtb = xin.tile([128, KT, B], BF16, name="xtb", tag="xtb")
            rwp = xin.tile([128, KT, 4 * E], BF16, name="rwp", tag="rwp")
            xrb = xin.tile([128, KT, B], BF16, name="xrb", tag="xrb")
            w00 = wpool.tile([128, KT, HD], BF16, name="w", tag="w")
            w01 = wpool.tile([128, KT, HD], BF16, name="w", tag="w")
            return xtb, rwp, xrb, w00, w01

        head = None
        for r in range(reps):
            # ---- SP ring: the input stream. Expert 0's first half rides
            # right behind xtb (split in two so the PE starts sooner); the
            # router inputs follow, then the rest of the weight stream.
            # For reps > 1, the NEXT rep's head is interleaved into this
            # rep's tail so the PE's boundary idle stays under the ~3.4 us
            # HAM re-throttle window. ----
            if head is None:
                head = make_head()
                xtb, rwp, xrb, w00, w01 = head
                nc.sync.dma_start(xtb[:], xtb_d[:])
                nc.sync.dma_start(w00[:, 0:4, :], whs_d[0, 0, :, 0:4, :])
                nc.sync.dma_start(w00[:, 4:8, :], whs_d[0, 0, :, 4:8, :])
                nc.sync.dma_start(rwp[:], rwp_d[:])
                nc.sync.dma_start(xrb[:], xrb_d[:])
                nc.sync.dma_start(w01[:], whs_d[0, 1])
            else:
                xtb, rwp, xrb, w00, w01 = head

            wts = [
                [
                    w00 if (e == 0 and h == 0)
                    else w01 if (e == 0 and h == 1)
                    else wpool.tile([128, KT, HD], BF16, name="w", tag="w")
                    for h in range(2)
                ]
                for e in range(EPC)
            ]
            for e in range(1, EPC):
                for h in range(2):
                    if e == EPC - 1 and h == 1:
                        continue
                    if (e, h) in ((EPC - 2, 1), (EPC - 1, 0)):
                        # half-chunks near the tail: the PE rides the wire
                        # frontier there, and finer arrival granularity keeps
                        # its idle bursts under the HAM re-throttle window
                        nc.sync.dma_start(
                            wts[e][h][:, 0:4, :], whs_d[e, h, :, 0:4, :]
                        )
                        nc.sync.dma_start(
                            wts[e][h][:, 4:8, :], whs_d[e, h, :, 4:8, :]
                        )
                    else:
                        nc.sync.dma_start(wts[e][h][:], whs_d[e, h])

            # next rep's head tiles + first DMAs, woven into this rep's tail
            nhead = make_head() if r + 1 < reps else None
            if nhead is not None:
                nc.sync.dma_start(nhead[0][:], xtb_d[:])
                nc.sync.dma_start(nhead[3][:, 0:4, :], whs_d[0, 0, :, 0:4, :])
            wl = wts[EPC - 1][1]
            if nhead is None:
                # final rep: tail chunks shrink so the very last matmuls
                # start (and the output leaves) as soon as possible
                nc.sync.dma_start(wl[:, 0:4, :], whs_d[EPC - 1, 1, :, 0:4, :])
                nc.sync.dma_start(wl[:, 4:6, :], whs_d[EPC - 1, 1, :, 4:6, :])
                nc.sync.dma_start(wl[:, 6:7, :], whs_d[EPC - 1, 1, :, 6:7, :])
                nc.sync.dma_start(wl[:, 7:8, :], whs_d[EPC - 1, 1, :, 7:8, :])
            else:
                # middle reps: keep boundary chunks big — many small
                # transfers shrink the bytes in flight across the 8 HWDGE
                # lanes and the wire sags right at the rep boundary
                nc.sync.dma_start(wl[:, 0:4, :], whs_d[EPC - 1, 1, :, 0:4, :])
                nc.sync.dma_start(nhead[3][:, 4:8, :], whs_d[0, 0, :, 4:8, :])
                nc.sync.dma_start(wl[:, 4:8, :], whs_d[EPC - 1, 1, :, 4:8, :])
            if nhead is not None:
                nc.sync.dma_start(nhead[1][:], rwp_d[:])
                nc.sync.dma_start(nhead[2][:], xrb_d[:])
                nc.sync.dma_start(nhead[4][:], whs_d[0, 1])
            head = nhead

            # ---- ACT ring: small inputs (outputs join it at the end) ----
            bscl = _ctile(const, "bscl", [EPC, D], BF16)
            nc.scalar.dma_start(bscl[:], bscl_d[:])
            sel = _ctile(const, "sel", [2 * E, 2 * EPC], F32)
            nc.scalar.dma_start(sel[:], sel_d[:])

            # ---- expert 0 first half: the earliest possible PE work ----
            pe0 = [ps_e.tile([B, HD], F32, name="pe") for _ in range(2)]
            for k in range(KT):
                nc.tensor.matmul(
                    pe0[0][:], xtb[:, k, :], wts[0][0][:, k, :],
                    start=(k == 0), stop=(k == KT - 1),
                )

            # ---- router logits, exact from bf16 pieces: all three product
            # chains accumulate into one PSUM region:
            # logits = sum_k xtb@rwb + xtb@rwr + xrb@rwb
            pl = ps_small.tile([B, 2 * E], F32, name="ps")
            for k in range(KT):
                nc.tensor.matmul(
                    pl[:], xtb[:, k, :], rwp[:, k, 0:2 * E],
                    start=(k == 0), stop=False,
                )
            for k in range(KT):
                nc.tensor.matmul(
                    pl[:], xtb[:, k, :], rwp[:, k, 2 * E:4 * E],
                    start=False, stop=False,
                )
            for k in range(KT):
                nc.tensor.matmul(
                    pl[:], xrb[:, k, :], rwp[:, k, 0:2 * E],
                    start=False, stop=(k == KT - 1),
                )
            logits = _ctile(const, "logits", [B, 2 * E], F32)
            nc.vector.tensor_copy(logits[:], pl[:])

            # ---- top-2 + softmax per half -> dense mix coeffs [B, 64].
            # The two halves' max/mask phases run first so one [B, 2]
            # ACTIVATE covers both exps — one ACT round-trip instead of two
            # in the serial chain that feeds the pinned transpose. ----
            mix_comb = _ctile(const, "mix_comb", [B, 2 * E], F32)
            dgap = _ctile(const, "dgap", [B, 2], F32)
            ed = _ctile(const, "ed", [B, 2], F32)
            m1s, m2s = [], []
            for h in range(2):
                lh = logits[:, h * E:(h + 1) * E]
                mx1 = _ctile(const, f"mx1_{h}", [B, 1], F32)
                nc.vector.tensor_reduce(mx1[:], lh, axis=mybir.AxisListType.X, op=ALU.max)
                m1 = _ctile(const, f"m1_{h}", [B, E], F32)
                nc.vector.tensor_scalar(m1[:], lh, mx1[:], None, op0=ALU.is_ge)
                msk = _ctile(const, f"msk_{h}", [B, E], F32)
                nc.vector.scalar_tensor_tensor(
                    msk[:], m1[:], -1e30, lh, op0=ALU.mult, op1=ALU.add
                )
                mx2 = _ctile(const, f"mx2_{h}", [B, 1], F32)
                nc.vector.tensor_reduce(mx2[:], msk[:], axis=mybir.AxisListType.X, op=ALU.max)
                m2 = _ctile(const, f"m2_{h}", [B, E], F32)
                nc.vector.tensor_scalar(m2[:], msk[:], mx2[:], None, op0=ALU.is_ge)
                nc.vector.tensor_sub(dgap[:, h:h + 1], mx2[:], mx1[:])
                m1s.append(m1)
                m2s.append(m2)
            nc.scalar.activation(ed[:], dgap[:], ACTF.Exp)
            for h in range(2):
                den = _ctile(const, f"den_{h}", [B, 1], F32)
                nc.vector.tensor_scalar_add(den[:], ed[:, h:h + 1], 1.0)
                p1 = _ctile(const, f"p1_{h}", [B, 1], F32)
                nc.vector.reciprocal(p1[:], den[:])
                p2 = _ctile(const, f"p2_{h}", [B, 1], F32)
                nc.vector.tensor_mul(p2[:], ed[:, h:h + 1], p1[:])
                t2 = _ctile(const, f"t2_{h}", [B, E], F32)
                nc.vector.tensor_scalar_mul(t2[:], m2s[h][:], p2[:])
                nc.vector.scalar_tensor_tensor(
                    mix_comb[:, h * E:(h + 1) * E], m1s[h][:], p1[:], t2[:],
                    op0=ALU.mult, op1=ALU.add,
                )


            # ---- expert 0 second half ----
            for k in range(KT):
                nc.tensor.matmul(
                    pe0[1][:], xtb[:, k, :], wts[0][1][:, k, :],
                    start=(k == 0), stop=(k == KT - 1),
                )

            # ---- expert 1 first half ----
            pe1 = [ps_e.tile([B, HD], F32, name="pe") for _ in range(2)]
            for k in range(KT):
                nc.tensor.matmul(
                    pe1[0][:], xtb[:, k, :], wts[1][0][:, k, :],
                    start=(k == 0), stop=(k == KT - 1),
                )

            # The mix-coefficient PE ops (transpose, per-core selection) must
            # sort AFTER the weight-fed expert matmul blocks in the in-order
            # PE queue: the scheduler's cost-model sim otherwise pulls them
            # right behind the router (it underestimates when the expert
            # weights land), and the transpose's softmax wait then stalls the
            # PE ~3.4 us — long enough to re-throttle the HAM. Priorities
            # can't fix this (ready work dispatches to a sim-idle engine
            # immediately), so pin it with a REAL dependency: the transpose's
            # identity operand is a no-op blend with expert 1's weight tile,
            # making the transpose sim-ready only after w10 — which slots it
            # behind e0h1/e1h0 in the PE queue. At runtime the blend costs
            # ~200 ns on DVE and resolves as soon as w10 lands.
            ident2 = _ctile(const, "ident2", [128, 128], F32)
            nc.vector.scalar_tensor_tensor(
                ident2[:], wts[1][1][:, 0, 0:128], 0.0, ident[:],
                op0=ALU.mult, op1=ALU.add,
            )
            ptm = ps_small.tile([2 * E, B], F32, name="ps")
            nc.tensor.transpose(ptm[:], mix_comb[:], ident2[:])
            mixT = _ctile(const, "mixT", [2 * E, B], F32)
            nc.vector.tensor_copy(mixT[:], ptm[:])
            if True:

                # one psum tile holds both tiny coefficient matmuls (disjoint
                # regions): weight-mix [B, 4] at cols 0:4, bias-mix [4, B]
                # (already transposed) at cols 4:4+B
                pml = ps_small.tile([B, EPC + B], F32, name="ps")
                nc.tensor.matmul(
                    pml[:, 0:EPC], mixT[:], sel[:, 0:EPC], start=True, stop=True
                )
                nc.tensor.matmul(
                    pml[0:EPC, EPC:EPC + B], sel[:, EPC:2 * EPC], mixT[:],
                    start=True, stop=True,
                )
                mix_loc = _ctile(const, "mix_loc", [B, EPC], F32)
                nc.vector.tensor_copy(mix_loc[:], pml[:, 0:EPC])
                mixbT = _ctile(const, "mixbT", [EPC, B], BF16)
                nc.vector.tensor_copy(mixbT[:], pml[0:EPC, EPC:EPC + B])

            # ---- expert 1 second half ----
            for k in range(KT):
                nc.tensor.matmul(
                    pe1[1][:], xtb[:, k, :], wts[1][1][:, k, :],
                    start=(k == 0), stop=(k == KT - 1),
                )

            # ---- local bias term: mixb_loc @ bscl -> [B, D] ----
            pb = [ps_e.tile([B, HD], F32, name="pe") for _ in range(2)]
            bias_sb = _ctile(const, "bias_sb", [B, D], F32)
            for h in range(2):
                nc.tensor.matmul(
                    pb[h][:], mixbT[:], bscl[:, h * HD:(h + 1) * HD],
                    start=True, stop=True,
                )
                nc.vector.tensor_copy(bias_sb[:, h * HD:(h + 1) * HD], pb[h][:])

            # ---- mix-accumulate chain: acc_e = pe_e * mix_e + acc_{e-1},
            # seeded with the bias ----
            acc0 = _ctile(const, "acc0", [B, D], F32)
            acc1 = _ctile(const, "acc1", [B, D], F32)
            for h in range(2):
                hs, he = h * HD, (h + 1) * HD
                nc.vector.scalar_tensor_tensor(
                    acc0[:, hs:he], pe0[h][:], mix_loc[:, 0:1],
                    bias_sb[:, hs:he], op0=ALU.mult, op1=ALU.add,
                )
                nc.vector.scalar_tensor_tensor(
                    acc1[:, hs:he], pe1[h][:], mix_loc[:, 1:2],
                    acc0[:, hs:he], op0=ALU.mult, op1=ALU.add,
                )

            # ---- experts 2 and 3 ----
            prev = acc1
            for e in range(2, EPC):
                last = e == EPC - 1
                pe = [ps_e.tile([B, HD], F32, name="pe") for _ in range(2)]
                # bf16 final accumulator: host sums the 8 partials in f64
                acc = _ctile(const, f"acc{e}", [B, D], BF16 if last else F32)
                for h in range(2):
                    hs, he = h * HD, (h + 1) * HD
                    for k in range(KT):
                        nc.tensor.matmul(
                            pe[h][:], xtb[:, k, :], wts[e][h][:, k, :],
                            start=(k == 0), stop=(k == KT - 1),
                        )
                    nc.vector.scalar_tensor_tensor(
                        acc[:, hs:he], pe[h][:], mix_loc[:, e:e + 1],
                        prev[:, hs:he], op0=ALU.mult, op1=ALU.add,
                    )
                    if last:
                        # middle reps: outputs ride SWDGE (gpsimd) — HWDGE
                        # completion lanes are shared round-robin by BOTH
                        # rings, so a compute-gated output DMA on a lane
                        # stalls the next rep's input DMA on that lane. The
                        # 8 DMA-SW lanes are a separate pool. The final rep
                        # has nothing left to stall, so its outputs take the
                        # lower-latency HWDGE path out.
                        oeng = nc.scalar if r == reps - 1 else nc.gpsimd
                        oeng.dma_start(out_d[:, hs:he], acc[:, hs:he])
                prev = acc

    nc.finalize()
    return nc


def make_input_maps(x, router_w, bias_router_w, expert_weights, expert_biases):
    xt = np.ascontiguousarray(
        x.T.reshape(KT, 128, B).transpose(1, 0, 2), dtype=np.float32
    )
    xtb = xt.astype(ml_dtypes.bfloat16)
    xrb = (xt - xtb.astype(np.float32)).astype(ml_dtypes.bfloat16)
    rw = (
        np.concatenate([router_w, bias_router_w], axis=1)
        .reshape(KT, 128, 2 * E)
        .transpose(1, 0, 2)
        .astype(np.float32)
    )
    rwb = rw.astype(ml_dtypes.bfloat16)
    rwr = (rw - rwb.astype(np.float32)).astype(ml_dtypes.bfloat16)
    rwp = np.ascontiguousarray(np.concatenate([rwb, rwr], axis=2))

    in_maps = []
    for c in range(NCORES):
        we = (
            expert_weights[c * EPC:(c + 1) * EPC]
            .reshape(EPC, KT, 128, 2, HD)
            .transpose(0, 3, 2, 1, 4)
        )
        whs = np.ascontiguousarray(we).astype(ml_dtypes.bfloat16)
        bscl = np.ascontiguousarray(
            expert_biases[c * EPC:(c + 1) * EPC]
        ).astype(ml_dtypes.bfloat16)
        selc = np.zeros((2 * E, 2 * EPC), dtype=np.float32)
        for j in range(EPC):
            selc[c * EPC + j, j] = 1.0
            selc[E + c * EPC + j, EPC + j] = 1.0
        in_maps.append(
            dict(xtb=xtb, xrb=xrb, rwp=rwp, whs=whs, bscl=bscl, sel=selc)
        )
    return in_maps


def kernel(x, router_w, bias_router_w, expert_weights, expert_biases, **bench_kwargs):
    in_maps = make_input_maps(x, router_w, bias_router_w, expert_weights, expert_biases)
    nc = build_program()
    res = run_bass_kernel_spmd(nc, in_maps, list(range(NCORES)), **bench_kwargs)
    out = np.zeros((B, D), dtype=np.float64)
    for r in res.results:
        out += r["out"].astype(np.float64)
    final = out.astype(np.float32)
    if bench_kwargs:
        kernel.last_result = res
    return final
